# revision 13
# baseline (speedup 1.0000x reference)
"""MultiHeadedAttention Trainium2 Bass kernel.

Full inputs in, full output out. Sharding: 8 cores = 4 batches x 2 head-pairs
(data-parallel over batch, tensor-parallel over the 4 heads). Each core:
  Q/K projections for its 2 heads      -> [128, 2048] (chan-major)
  V projection directly transposed     -> vt [m, d] with a ones column
  per head: scoresT[m,n] = K^T Q, exp (scale=1/8, no max-sub: |s/8| < ~5),
  xT[n, d+1] accumulated over m in PSUM (ones col gives softmax sums),
  per-partition normalize, PE-transpose back to x[d, n],
  out projection with both heads accumulated in PSUM.
Host sums the two per-batch partials and adds the output bias.
"""

import sys

if "/opt/trn_rl_repo" not in sys.path:
    sys.path.insert(0, "/opt/trn_rl_repo")

import numpy as np

B, D, N, H = 4, 256, 2048, 4
DIM = D // H  # 64
NW = 4  # 512-wide n windows
MB = 16  # 128-wide m blocks

_CACHE = {}


def _emit(ctx, tc, io):
    import concourse.mybir as mybir
    from concourse.masks import make_identity

    nc = tc.nc
    f32 = mybir.dt.float32
    f32r = mybir.dt.float32r
    bf16 = mybir.dt.bfloat16
    EXP = mybir.ActivationFunctionType.Exp

    const = ctx.enter_context(tc.tile_pool(name="const", bufs=1))
    xin = ctx.enter_context(tc.tile_pool(name="xin", bufs=4))
    big = ctx.enter_context(tc.tile_pool(name="big", bufs=1))
    xpool = ctx.enter_context(tc.tile_pool(name="xpool", bufs=2))
    pb = ctx.enter_context(tc.tile_pool(name="probs", bufs=2))
    work = ctx.enter_context(tc.tile_pool(name="work", bufs=4))
    outp = ctx.enter_context(tc.tile_pool(name="outp", bufs=3))
    psA = ctx.enter_context(tc.tile_pool(name="psA", bufs=2, space="PSUM"))
    psX = ctx.enter_context(tc.tile_pool(name="psX", bufs=3, space="PSUM"))

    # ---- constants / weights ----
    wqt_sb = const.tile([128, 2, 128], f32r, tag="wqt")
    nc.sync.dma_start(wqt_sb, io["wqt"].rearrange("(c p) o -> p c o", p=128))
    wkt_sb = const.tile([128, 2, 128], f32r, tag="wkt")
    nc.sync.dma_start(wkt_sb, io["wkt"].rearrange("(c p) o -> p c o", p=128))
    wvt_sb = const.tile([128, 2, 128], f32, tag="wvt")
    nc.sync.dma_start(wvt_sb, io["wvt"].rearrange("(c p) o -> p c o", p=128))
    wmt0_sb = const.tile([64, 256], f32r, tag="wmt0")
    nc.sync.dma_start(wmt0_sb, io["wmt0"])
    wmt1_sb = const.tile([64, 256], f32r, tag="wmt1")
    nc.sync.dma_start(wmt1_sb, io["wmt1"])
    bq_sb = const.tile([128, 1], f32, tag="bq")
    nc.sync.dma_start(bq_sb, io["bq"])
    bk_sb = const.tile([128, 1], f32, tag="bk")
    nc.sync.dma_start(bk_sb, io["bk"])
    bv_sb = const.tile([1, 128], f32, tag="bv")
    nc.sync.dma_start(bv_sb, io["bv"])

    # bf16 copies for the value path
    wvtb = const.tile([128, 2, 128], bf16, tag="wvtb")
    nc.gpsimd.tensor_copy(wvtb, wvt_sb)
    bvb = const.tile([1, 128], bf16, tag="bvb")
    nc.gpsimd.tensor_copy(bvb, bv_sb)
    onesb = const.tile([1, 128], bf16, tag="onesb")
    nc.gpsimd.memset(onesb, 1.0)
    identb = const.tile([128, 128], bf16, tag="identb")
    make_identity(nc, identb)

    # ---- heavy input loads (chunked for DMA spread) ----
    xq_t, xk_t, xv_t, xvb_t = [], [], [], []
    for w in range(NW):
        s = slice(w * 512, (w + 1) * 512)
        tq = xin.tile([128, 2, 512], f32r, tag="xq")
        nc.sync.dma_start(tq, io["xq"].rearrange("(c p) n -> p c n", p=128)[:, :, s])
        xq_t.append(tq)
        tk = xin.tile([128, 2, 512], f32r, tag="xk")
        nc.sync.dma_start(tk, io["xk"].rearrange("(c p) n -> p c n", p=128)[:, :, s])
        xk_t.append(tk)
        tv = xin.tile([128, 2, 512], f32, tag="xv")
        nc.sync.dma_start(tv, io["xv"].rearrange("(c p) n -> p c n", p=128)[:, :, s])
        xv_t.append(tv)
        tvb = xin.tile([128, 2, 512], bf16, tag="xvb")
        nc.gpsimd.tensor_copy(tvb, tv)
        xvb_t.append(tvb)

    # ---- phase 1: Q and K projections -> [128, 2048] (64 rows per head) ----
    q_sb = big.tile([128, 2048], f32r, tag="q")
    k_sb = big.tile([128, 2048], f32r, tag="k")
    for xt, wt, bias, dst in ((xq_t, wqt_sb, bq_sb, q_sb), (xk_t, wkt_sb, bk_sb, k_sb)):
        for w in range(NW):
            ps = psA.tile([128, 1024], f32, tag="ps")
            nc.tensor.matmul(
                ps[:, 0:512],
                lhsT=wt[:, 0, :],
                rhs=xt[w][:, 0, :],
                start=True,
                stop=False,
            )
            nc.tensor.matmul(
                ps[:, 0:512],
                lhsT=wt[:, 1, :],
                rhs=xt[w][:, 1, :],
                start=False,
                stop=True,
            )
            nc.vector.tensor_scalar_add(dst[:, w * 512 : (w + 1) * 512], ps[:, 0:512], bias)

    # ---- phase 2: V^T with bias (both heads) -> vt[m, (h, d+ones)] bf16 ----
    vt = big.tile([128, MB, 2, 65], bf16, tag="vt")
    nc.gpsimd.memset(vt[:, :, :, 64:65], 1.0)
    for mb in range(MB):
        w, off = divmod(mb, 4)
        ms = slice(off * 128, (off + 1) * 128)
        ps = psA.tile([128, 1024], f32, tag="ps")
        pvt = ps[:, 0:128]
        nc.tensor.matmul(pvt, lhsT=onesb, rhs=bvb, start=True, stop=False)
        nc.tensor.matmul(pvt, lhsT=xvb_t[w][:, 0, ms], rhs=wvtb[:, 0, :], start=False, stop=False)
        nc.tensor.matmul(pvt, lhsT=xvb_t[w][:, 1, ms], rhs=wvtb[:, 1, :], start=False, stop=True)
        # [128, 128] fp32 psum -> [128, 2, 64] bf16 slices of vt
        nc.vector.tensor_copy(vt[:, mb, :, 0:64], pvt.rearrange("m (h d) -> m h d", h=2))

    # ---- phase 3: attention per head ----
    x_sb = []
    for h in range(2):
        qh = q_sb[h * 64 : (h + 1) * 64, :]
        kh = k_sb[h * 64 : (h + 1) * 64, :]
        px = [psX.tile([128, 7, 65], f32, tag="px", name=f"px{h}_{i}") for i in range(3)]

        def pxi(j):
            t = min(j // 7, 2)
            return px[t][:, j - 7 * t, :]

        for mb in range(MB):
            pt = pb.tile([128, 2048], bf16, tag="pt")
            for nh in range(2):
                sc = psA.tile([128, 1024], f32, tag="ps")
                for s2 in range(2):
                    n0 = nh * 1024 + s2 * 512
                    nc.tensor.matmul(
                        sc[:, s2 * 512 : (s2 + 1) * 512],
                        lhsT=kh[:, mb * 128 : (mb + 1) * 128],
                        rhs=qh[:, n0 : n0 + 512],
                        start=True,
                        stop=True,
                    )
                nc.scalar.activation(pt[:, nh * 1024 : (nh + 1) * 1024], sc, EXP, scale=0.125)
            for j in range(MB):
                # start=True zeroes the whole 2KB PSUM bank, so only the first
                # slice written into each px bank may set it (at mb==0).
                jj0 = j in (0, 7, 14)
                nc.tensor.matmul(
                    pxi(j),
                    lhsT=pt[:, j * 128 : (j + 1) * 128],
                    rhs=vt[:, mb, h, :],
                    start=(mb == 0 and jj0),
                    stop=(mb == MB - 1),
                    skip_group_check=True,
                )
            if h == 0 and "dbg_pt" in io:
                nc.sync.dma_start(io["dbg_pt"][mb], pt)

        # softmax sums -> reciprocal (per-partition n)
        rin = work.tile([128, 16], f32, tag="rin")
        nc.vector.tensor_copy(rin[:, 0:7], px[0][:, :, 64])
        nc.vector.tensor_copy(rin[:, 7:14], px[1][:, :, 64])
        nc.vector.tensor_copy(rin[:, 14:16], px[2][:, 0:2, 64])
        r = work.tile([128, 16], f32, tag="r")
        nc.vector.reciprocal(r, rin)

        # normalize + cast bf16
        xtn = xpool.tile([128, 16, 64], bf16, tag="xtn")
        for j in range(MB):
            nc.vector.tensor_scalar_mul(xtn[:, j, :], pxi(j)[:, 0:64], r[:, j : j + 1])
        if h == 0 and "dbg_xtn" in io:
            nc.sync.dma_start(io["dbg_xtn"], xtn)
            nc.sync.dma_start(io["dbg_r"], r)

        # transpose xT -> x [64, 2048] f32 via identity matmul
        xh = xpool.tile([64, 2048], f32r, tag="x")
        for g in range(4):
            xt_ps = psA.tile([128, 1024], f32, tag="ps")
            for jj in range(4):
                j = g * 4 + jj
                # two transposes share each 2KB bank; start only on the first
                nc.tensor.matmul(
                    xt_ps[0:64, jj * 128 : (jj + 1) * 128],
                    lhsT=xtn[:, j, :],
                    rhs=identb,
                    start=(jj % 2 == 0),
                    stop=(jj % 2 == 1),
                    skip_group_check=True,
                )
            nc.vector.tensor_copy(xh[:, g * 512 : (g + 1) * 512], xt_ps[0:64, 0:512])
        x_sb.append(xh)

    if "dbg_q" in io:
        nc.sync.dma_start(io["dbg_q"], q_sb)
        nc.sync.dma_start(io["dbg_k"], k_sb)
        nc.sync.dma_start(io["dbg_vt"], vt)
        nc.sync.dma_start(io["dbg_x0"], x_sb[0])
        nc.sync.dma_start(io["dbg_x1"], x_sb[1])

    # ---- phase 4: out projection, heads accumulated in PSUM ----
    for oc in range(2):
        for w in range(NW):
            ps = psA.tile([128, 1024], f32, tag="ps")
            po = ps[:, 0:512]
            ws = slice(w * 512, (w + 1) * 512)
            ocs = slice(oc * 128, (oc + 1) * 128)
            nc.tensor.matmul(
                po,
                lhsT=wmt0_sb[:, ocs],
                rhs=x_sb[0][:, ws],
                start=True,
                stop=False,
            )
            nc.tensor.matmul(
                po,
                lhsT=wmt1_sb[:, ocs],
                rhs=x_sb[1][:, ws],
                start=False,
                stop=True,
            )
            ot = outp.tile([128, 512], f32, tag="ot")
            nc.vector.tensor_copy(ot, po)
            nc.sync.dma_start(io["out"][ocs, ws], ot)


def _build_nc(debug_dumps=False):
    key = ("nc", debug_dumps)
    if key in _CACHE:
        return _CACHE[key]
    from contextlib import ExitStack

    import concourse.mybir as mybir
    import concourse.tile as tile
    from concourse import bacc

    f32 = mybir.dt.float32
    nc = bacc.Bacc("TRN2", target_bir_lowering=False, debug=False, num_devices=8)
    f32r = mybir.dt.float32r
    io = {}
    for name, shape, dt_ in (
        ("xq", [256, 2048], f32r),
        ("xk", [256, 2048], f32r),
        ("xv", [256, 2048], f32),
        ("wqt", [256, 128], f32r),
        ("wkt", [256, 128], f32r),
        ("wvt", [256, 128], f32),
        ("bq", [128, 1], f32),
        ("bk", [128, 1], f32),
        ("bv", [1, 128], f32),
        ("wmt0", [64, 256], f32r),
        ("wmt1", [64, 256], f32r),
    ):
        io[name] = nc.dram_tensor(name, shape, dt_, kind="ExternalInput").ap()
    io["out"] = nc.dram_tensor("out", [256, 2048], f32, kind="ExternalOutput").ap()
    if debug_dumps:
        bf16 = mybir.dt.bfloat16
        io["dbg_q"] = nc.dram_tensor("dbg_q", [128, 2048], f32, kind="ExternalOutput").ap()
        io["dbg_k"] = nc.dram_tensor("dbg_k", [128, 2048], f32, kind="ExternalOutput").ap()
        io["dbg_vt"] = nc.dram_tensor("dbg_vt", [128, MB, 2, 65], bf16, kind="ExternalOutput").ap()
        io["dbg_x0"] = nc.dram_tensor("dbg_x0", [64, 2048], f32, kind="ExternalOutput").ap()
        io["dbg_x1"] = nc.dram_tensor("dbg_x1", [64, 2048], f32, kind="ExternalOutput").ap()
        io["dbg_pt"] = nc.dram_tensor("dbg_pt", [MB, 128, 2048], bf16, kind="ExternalOutput").ap()
        io["dbg_xtn"] = nc.dram_tensor("dbg_xtn", [128, 16, 64], bf16, kind="ExternalOutput").ap()
        io["dbg_r"] = nc.dram_tensor("dbg_r", [128, 16], f32, kind="ExternalOutput").ap()

    with tile.TileContext(nc) as tc:
        with ExitStack() as ctx:
            _emit(ctx, tc, io)
    nc.compile()
    _CACHE[key] = nc
    _CACHE[(key, "io")] = io
    return nc


def _round_f32r(a):
    """Round fp32 to the fp32r format (11 mantissa bits, round-to-nearest-even)."""
    u = np.ascontiguousarray(a, np.float32).view(np.uint32).copy()
    u += np.uint32(0x7FF) + ((u >> np.uint32(12)) & np.uint32(1))
    u &= np.uint32(0xFFFFF000)
    return u.view(np.float32)


_F32R_KEYS = ("xq", "xk", "wqt", "wkt", "wmt0", "wmt1")


def make_in_maps(query, key, value, wq, bq, wk, bk, wv, bv, wm, bm):
    arrs = {}
    f = lambda a: np.ascontiguousarray(np.asarray(a), dtype=np.float32)
    query, key, value = f(query), f(key), f(value)
    wq, wk, wv, wm = f(wq), f(wk), f(wv), f(wm)
    bq, bk, bv = f(bq), f(bk), f(bv)
    in_maps = []
    for c in range(8):
        b, pair = divmod(c, 2)
        hs = (2 * pair, 2 * pair + 1)
        idx = np.array([d * H + h for h in hs for d in range(DIM)])
        m = {
            "xq": query[b],
            "xk": key[b],
            "xv": value[b],
            "wqt": f(wq[idx].T),
            "wkt": f(wk[idx].T),
            "wvt": f(wv[idx].T),
            "bq": bq[idx].reshape(128, 1),
            "bk": bk[idx].reshape(128, 1),
            "bv": bv[idx].reshape(1, 128),
            "wmt0": f(wm[:, idx[:64]].T),
            "wmt1": f(wm[:, idx[64:]].T),
        }
        in_maps.append(
            {k: (_round_f32r(v) if k in _F32R_KEYS else f(v)) for k, v in m.items()}
        )
    return in_maps


def run(in_maps, trace=False, **kw):
    from concourse import bass_utils

    nc = _build_nc()
    return bass_utils.run_bass_kernel_spmd(
        nc, in_maps, core_ids=list(range(8)), trace=trace, **kw
    )


def gather(results, bm):
    bm = np.asarray(bm, dtype=np.float32)
    outs = [np.asarray(r["out"], dtype=np.float32) for r in results]
    return np.stack([outs[2 * b] + outs[2 * b + 1] + bm[:, None] for b in range(B)])


def kernel(query, key, value, wq, bq, wk, bk, wv, bv, wm, bm):
    in_maps = make_in_maps(query, key, value, wq, bq, wk, bk, wv, bv, wm, bm)
    res = run(in_maps)
    return gather(res.results, bm)
